# revision 7
# baseline (speedup 1.0000x reference)
"""ChebyKAN layer kernel for 8 Trainium2 NeuronCores.

y[t, o] = sum_{i,d} T_d(tanh(x[t, i])) * coeffs[i, o, d],  d = 0..8

Strategy (data-parallel over the 8192-token dim, 1024 tokens/core):
  - Host: transpose each core's x shard to [i, t] layout; fold the d=0 term
    (T_0 == 1) into a per-output bias vector; scale coeffs by 2^14 and cast
    to fp16 (unscaled they are ~3e-6 — subnormal in fp16); pre-arrange the
    d>=1 coeffs as 64 K-tiles of [128 i, 1024 o].
  - Device: tanh on ScalarE, Chebyshev recurrence T_d = 2 t T_{d-1} - T_{d-2}
    in fp32 on VectorE, convert each T_d to fp16 on ScalarE, then a
    K=8192 fp16 matmul (coeff tile stationary, basis tile moving) with fp32
    PSUM accumulation. PSUM drain on ScalarE fuses the 2^-14 descale and the
    d=0 bias: y = Copy(psum * 2^-14 + bias).
  - Output is produced transposed per core ([o, t]); the host gather
    transposes back.

fp16 multiplies are exact on the PE (verified on HW), so the only precision
loss is the fp16 rounding of the two operands: ~2.5e-4 max relative error
against the fp32 reference.
"""

import numpy as np

N_CORES = 8
N_TOKENS = 8192
NI = 1024
NO = 1024
DEG = 8  # degree+1 = 9 basis functions, d=0 folded into bias
TOK_PER_CORE = N_TOKENS // N_CORES  # 1024
TT = 512  # token tile (PSUM free dim)
NTT = TOK_PER_CORE // TT  # 2
IB = NI // 128  # 8 i-blocks
OB = NO // 128  # 8 o-blocks
KT = IB * DEG  # 64 K-tiles of 128
SCALE = 2.0 ** 14

_CACHE = {}


def _install_ntff_hook_shim():
    """The agent image's antenv lacks axon_hooks, so the boot path silently
    skipped registering the NTFF profile hook. Recreate it so trace=True
    works when test harnesses want timing. Harmless if unused."""
    import sys
    import types

    if "antenv.axon_hooks" in sys.modules:
        return
    mod = types.ModuleType("antenv.axon_hooks")
    mod._hook = None
    mod.set_axon_ntff_profile_hook = lambda h: setattr(mod, "_hook", h)
    mod.get_axon_ntff_profile_hook = lambda: mod._hook
    sys.modules["antenv.axon_hooks"] = mod
    try:
        import antenv

        antenv.axon_hooks = mod
    except ImportError:
        pass
    try:
        from trn_agent_boot.trn_boot import _ntff_profile_via_ctypes

        hook = _ntff_profile_via_ctypes("/opt/axon/libaxon_pjrt.so")
        if hook is not None:
            mod._hook = hook
    except Exception:
        pass


def _build():
    if "nc" in _CACHE:
        return _CACHE["nc"]

    _install_ntff_hook_shim()

    import concourse.bacc as bacc
    import concourse.mybir as mybir
    import concourse.tile as tile

    AF = mybir.ActivationFunctionType
    ALU = mybir.AluOpType
    f32 = mybir.dt.float32
    f16 = mybir.dt.float16

    nc = bacc.Bacc()
    xt_ext = nc.declare_dram_parameter("xt", [NI, TOK_PER_CORE], f32, isOutput=False)
    wk_ext = nc.declare_dram_parameter("wk", [KT, 128, NO], f16, isOutput=False)
    bias_ext = nc.declare_dram_parameter("bias", [OB, 128, 1], f32, isOutput=False)
    yt_ext = nc.declare_dram_parameter("yt", [NO, TOK_PER_CORE], f32, isOutput=True)

    with tile.TileContext(nc) as tc:
        with (
            tc.tile_pool(name="xpool", bufs=3) as xpool,
            tc.tile_pool(name="tpool", bufs=3) as tpool,
            tc.tile_pool(name="fpool", bufs=8) as fpool,
            tc.tile_pool(name="bpool", bufs=72) as bpool,
            tc.tile_pool(name="wpool", bufs=4) as wpool,
            tc.tile_pool(name="pspool", bufs=8, space="PSUM") as pspool,
            tc.tile_pool(name="opool", bufs=4) as opool,
            tc.tile_pool(name="biaspool", bufs=OB) as biaspool,
        ):
            bias_tiles = None

            for tt in range(NTT):
                psum = [
                    pspool.tile([128, TT], f32, tag="psum", name=f"psum_{tt}_{ob}")
                    for ob in range(OB)
                ]
                for ib in range(IB):
                    # ---- basis for this (ib, tt): T_1..T_8 as fp16 tiles ----
                    xtile = xpool.tile([128, TT], f32, tag="x")
                    nc.sync.dma_start(
                        out=xtile,
                        in_=xt_ext[ib * 128 : (ib + 1) * 128, tt * TT : (tt + 1) * TT],
                    )
                    t_f = tpool.tile([128, TT], f32, tag="t")
                    nc.scalar.activation(out=t_f, in_=xtile, func=AF.Tanh)

                    basis = []
                    b1 = bpool.tile([128, TT], f16, tag="basis")
                    nc.scalar.activation(out=b1, in_=t_f, func=AF.Copy)
                    basis.append(b1)

                    sq = fpool.tile([128, TT], f32, tag="frec")
                    nc.scalar.activation(out=sq, in_=t_f, func=AF.Square)
                    t2 = fpool.tile([128, TT], f32, tag="frec")
                    # T_2 = 2 t^2 - 1
                    nc.vector.tensor_scalar(
                        out=t2, in0=sq, scalar1=2.0, scalar2=1.0,
                        op0=ALU.mult, op1=ALU.subtract,
                    )
                    b2 = bpool.tile([128, TT], f16, tag="basis")
                    nc.scalar.activation(out=b2, in_=t2, func=AF.Copy)
                    basis.append(b2)

                    t_prev2, t_prev1 = t_f, t2
                    for d in range(3, DEG + 1):
                        p = fpool.tile([128, TT], f32, tag="frec")
                        nc.vector.tensor_tensor(
                            out=p, in0=t_f, in1=t_prev1, op=ALU.mult
                        )
                        t_cur = fpool.tile([128, TT], f32, tag="frec")
                        # T_d = 2 p - T_{d-2}
                        nc.vector.scalar_tensor_tensor(
                            out=t_cur, in0=p, scalar=2.0, in1=t_prev2,
                            op0=ALU.mult, op1=ALU.subtract,
                        )
                        bd = bpool.tile([128, TT], f16, tag="basis")
                        nc.scalar.activation(out=bd, in_=t_cur, func=AF.Copy)
                        basis.append(bd)
                        t_prev2, t_prev1 = t_prev1, t_cur

                    # ---- matmul accumulation over this ib's 8 degrees ----
                    # coeff K-tiles batched 4-per-DMA, except the very first
                    # i-block where single-kt DMAs shorten the kernel head
                    wts = {}
                    if tt == 0 and ib == 0:
                        for di in range(DEG):
                            wt1 = wpool.tile([128, 1, NO], f16, tag="w1")
                            nc.sync.dma_start(
                                out=wt1,
                                in_=wk_ext[di : di + 1].rearrange("k p o -> p k o"),
                            )
                            wts[di] = (wt1, 0)
                    else:
                        for dj in range(0, DEG, 4):
                            wt = wpool.tile([128, 4, NO], f16, tag="w")
                            nc.sync.dma_start(
                                out=wt,
                                in_=wk_ext[ib * DEG + dj : ib * DEG + dj + 4].rearrange(
                                    "k p o -> p k o"
                                ),
                            )
                            for di in range(dj, dj + 4):
                                wts[di] = (wt, di - dj)

                    if ib < IB - 1:
                        order = [(di, ob) for di in range(DEG) for ob in range(OB)]
                    else:
                        # last i-block: ob-major so PSUM banks complete
                        # staggered and drains overlap the tail of the stream
                        order = [(di, ob) for ob in range(OB) for di in range(DEG)]
                    for di, ob in order:
                        wt, wi = wts[di]
                        nc.tensor.matmul(
                            psum[ob],
                            wt[:, wi, ob * 128 : (ob + 1) * 128],
                            basis[di],
                            start=(ib == 0 and di == 0),
                            stop=(ib == IB - 1 and di == DEG - 1),
                        )

                if bias_tiles is None:
                    # emitted late: off the kernel-head critical path
                    bias_tiles = []
                    for ob in range(OB):
                        bt = biaspool.tile([128, 1], f32, tag="bias")
                        nc.sync.dma_start(out=bt, in_=bias_ext[ob])
                        bias_tiles.append(bt)

                # ---- drain: y = psum * 2^-14 + bias ----
                for ob in range(OB):
                    ot = opool.tile([128, TT], f32, tag="o")
                    nc.scalar.activation(
                        out=ot, in_=psum[ob], func=AF.Identity,
                        scale=float(1.0 / SCALE), bias=bias_tiles[ob],
                    )
                    nc.sync.dma_start(
                        out=yt_ext[ob * 128 : (ob + 1) * 128, tt * TT : (tt + 1) * TT],
                        in_=ot,
                    )

    nc.finalize()
    _CACHE["nc"] = nc
    return nc


def _prep_inputs(x, cheby_coeffs):
    x = np.asarray(x, dtype=np.float32)
    coeffs = np.asarray(cheby_coeffs, dtype=np.float32)

    bias = coeffs[:, :, 0].sum(axis=0).astype(np.float32)  # [NO]
    bias = np.ascontiguousarray(bias.reshape(OB, 128, 1))

    # wk[kt=(ib, d-1)][i_in][o] = coeffs[ib*128+i_in, o, d] * SCALE  (fp16)
    w = coeffs[:, :, 1:]  # [NI, NO, DEG]
    w = np.transpose(w.reshape(IB, 128, NO, DEG), (0, 3, 1, 2))  # [IB, DEG, 128, NO]
    wk = np.ascontiguousarray((w * SCALE).reshape(KT, 128, NO)).astype(np.float16)

    in_maps = []
    for c in range(N_CORES):
        xs = x[c * TOK_PER_CORE : (c + 1) * TOK_PER_CORE]  # [1024, NI]
        xt = np.ascontiguousarray(xs.T)  # [NI, 1024]
        in_maps.append({"xt": xt, "wk": wk, "bias": bias})
    return in_maps


def _gather(results):
    y = np.empty((N_TOKENS, NO), dtype=np.float32)
    for c in range(N_CORES):
        y[c * TOK_PER_CORE : (c + 1) * TOK_PER_CORE] = results[c]["yt"].T
    return y


def kernel(x, cheby_coeffs, _trace=False):
    from concourse.bass_utils import run_bass_kernel_spmd

    nc = _build()
    in_maps = _prep_inputs(x, cheby_coeffs)
    res = run_bass_kernel_spmd(
        nc, in_maps, list(range(N_CORES)), trace=_trace,
        **({"trace_cores": list(range(N_CORES))} if _trace else {}),
    )
    y = _gather(res.results)
    if _trace:
        return y, res
    return y


# revision 10
# speedup vs baseline: 1.2125x; 1.2125x over previous
"""ChebyKAN layer kernel for 8 Trainium2 NeuronCores.

y[t, o] = sum_{i,d} T_d(tanh(x[t, i])) * coeffs[i, o, d],  d = 0..8

Strategy (data-parallel over the 8192-token dim, 1024 tokens/core):
  - Host: transpose each core's x shard to [i, t] layout; fold the d=0 term
    (T_0 == 1) into a per-output bias vector; scale coeffs by 2^14 and cast
    to fp16 (unscaled they are ~3e-6 — subnormal in fp16); pre-arrange the
    d>=1 coeffs as 64 K-tiles of [128 i, 1024 o].
  - Device: tanh on ScalarE, Chebyshev recurrence T_d = 2 t T_{d-1} - T_{d-2}
    in fp32 on VectorE, convert each T_d to fp16 on ScalarE, then a
    K=8192 fp16 matmul (coeff tile stationary, basis tile moving) with fp32
    PSUM accumulation. PSUM drain on ScalarE fuses the 2^-14 descale and the
    d=0 bias: y = Copy(psum * 2^-14 + bias).
  - Output is produced transposed per core ([o, t]); the host gather
    transposes back.

fp16 multiplies are exact on the PE (verified on HW), so the only precision
loss is the fp16 rounding of the two operands: ~2.5e-4 max relative error
against the fp32 reference.
"""

import numpy as np

N_CORES = 8
N_TOKENS = 8192
NI = 1024
NO = 1024
DEG = 8  # degree+1 = 9 basis functions, d=0 folded into bias
TOK_PER_CORE = N_TOKENS // N_CORES  # 1024
TT = 512  # token tile (PSUM free dim)
NTT = TOK_PER_CORE // TT  # 2
IB = NI // 128  # 8 i-blocks
OB = NO // 128  # 8 o-blocks
KT = IB * DEG  # 64 K-tiles of 128
SCALE = 2.0 ** 14

_CACHE = {}


def _install_ntff_hook_shim():
    """The agent image's antenv lacks axon_hooks, so the boot path silently
    skipped registering the NTFF profile hook. Recreate it so trace=True
    works when test harnesses want timing. Harmless if unused."""
    import sys
    import types

    if "antenv.axon_hooks" in sys.modules:
        return
    mod = types.ModuleType("antenv.axon_hooks")
    mod._hook = None
    mod.set_axon_ntff_profile_hook = lambda h: setattr(mod, "_hook", h)
    mod.get_axon_ntff_profile_hook = lambda: mod._hook
    sys.modules["antenv.axon_hooks"] = mod
    try:
        import antenv

        antenv.axon_hooks = mod
    except ImportError:
        pass
    try:
        from trn_agent_boot.trn_boot import _ntff_profile_via_ctypes

        hook = _ntff_profile_via_ctypes("/opt/axon/libaxon_pjrt.so")
        if hook is not None:
            mod._hook = hook
    except Exception:
        pass


def _build():
    if "nc" in _CACHE:
        return _CACHE["nc"]

    _install_ntff_hook_shim()

    import concourse.bacc as bacc
    import concourse.mybir as mybir
    import concourse.tile as tile

    AF = mybir.ActivationFunctionType
    ALU = mybir.AluOpType
    f32 = mybir.dt.float32
    f16 = mybir.dt.float16

    nc = bacc.Bacc()
    xt_ext = nc.declare_dram_parameter("xt", [NI, TOK_PER_CORE], f32, isOutput=False)
    wk_ext = nc.declare_dram_parameter("wk", [KT, 128, NO], f16, isOutput=False)
    bias_ext = nc.declare_dram_parameter("bias", [OB, 128, 1], f32, isOutput=False)
    yt_ext = nc.declare_dram_parameter("yt", [NO, TOK_PER_CORE], f32, isOutput=True)

    with tile.TileContext(nc) as tc:
        with (
            tc.tile_pool(name="xpool", bufs=3) as xpool,
            tc.tile_pool(name="tpool", bufs=3) as tpool,
            tc.tile_pool(name="fpool", bufs=12) as fpool,
            tc.tile_pool(name="bpool", bufs=72) as bpool,
            tc.tile_pool(name="wpool", bufs=4) as wpool,
            tc.tile_pool(name="pspool", bufs=8, space="PSUM") as pspool,
            tc.tile_pool(name="opool", bufs=4) as opool,
            tc.tile_pool(name="biaspool", bufs=OB) as biaspool,
        ):
            bias_tiles = None

            for tt in range(NTT):
                psum = [
                    pspool.tile([128, TT], f32, tag="psum", name=f"psum_{tt}_{ob}")
                    for ob in range(OB)
                ]
                for ib in range(IB):
                    # ---- basis for this (ib, tt): T_1..T_8 as fp16 tiles ----
                    xtile = xpool.tile([128, TT], f32, tag="x")
                    nc.sync.dma_start(
                        out=xtile,
                        in_=xt_ext[ib * 128 : (ib + 1) * 128, tt * TT : (tt + 1) * TT],
                    )
                    t_f = tpool.tile([128, TT], f32, tag="t")
                    nc.scalar.activation(out=t_f, in_=xtile, func=AF.Tanh)

                    # T_2k = 2 T_k^2 - 1 (ACT Square + DVE tensor_scalar, both
                    # cheap); T_{2k+1} = 2 T_k T_{k+1} - t (DVE mult + STT).
                    # All in f32; each T_d converted to fp16 on DVE (2x mode).
                    T = {1: t_f}
                    basis = []
                    b1 = bpool.tile([128, TT], f16, tag="basis", name=f"b1_{tt}_{ib}")
                    nc.vector.tensor_copy(b1, t_f)
                    basis.append(b1)
                    for d in range(2, DEG + 1):
                        t_cur = fpool.tile([128, TT], f32, tag="frec", name=f"T{d}_{tt}_{ib}")
                        if d % 2 == 0:
                            sq = fpool.tile([128, TT], f32, tag="frec", name=f"sq{d}_{tt}_{ib}")
                            nc.scalar.activation(out=sq, in_=T[d // 2], func=AF.Square)
                            nc.vector.tensor_scalar(
                                out=t_cur, in0=sq, scalar1=2.0, scalar2=1.0,
                                op0=ALU.mult, op1=ALU.subtract,
                            )
                        else:
                            p = fpool.tile([128, TT], f32, tag="frec", name=f"p{d}_{tt}_{ib}")
                            nc.vector.tensor_tensor(
                                out=p, in0=T[d // 2], in1=T[d // 2 + 1], op=ALU.mult
                            )
                            nc.vector.scalar_tensor_tensor(
                                out=t_cur, in0=p, scalar=2.0, in1=t_f,
                                op0=ALU.mult, op1=ALU.subtract,
                            )
                        T[d] = t_cur
                        bd = bpool.tile([128, TT], f16, tag="basis", name=f"b{d}_{tt}_{ib}")
                        nc.vector.tensor_copy(bd, t_cur)
                        basis.append(bd)

                    # ---- matmul accumulation over this ib's 8 degrees ----
                    # coeff K-tiles batched 4-per-DMA, except the very first
                    # i-block where single-kt DMAs shorten the kernel head
                    wts = {}
                    if tt == 0 and ib == 0:
                        for di in range(DEG):
                            wt1 = wpool.tile([128, 1, NO], f16, tag="w1")
                            nc.sync.dma_start(
                                out=wt1,
                                in_=wk_ext[di : di + 1].rearrange("k p o -> p k o"),
                            )
                            wts[di] = (wt1, 0)
                    else:
                        for dj in range(0, DEG, 4):
                            wt = wpool.tile([128, 4, NO], f16, tag="w")
                            nc.sync.dma_start(
                                out=wt,
                                in_=wk_ext[ib * DEG + dj : ib * DEG + dj + 4].rearrange(
                                    "k p o -> p k o"
                                ),
                            )
                            for di in range(dj, dj + 4):
                                wts[di] = (wt, di - dj)

                    if ib < IB - 1:
                        order = [(di, ob) for di in range(DEG) for ob in range(OB)]
                    else:
                        # last i-block: ob-major so PSUM banks complete
                        # staggered and drains overlap the tail of the stream
                        order = [(di, ob) for ob in range(OB) for di in range(DEG)]
                    for di, ob in order:
                        wt, wi = wts[di]
                        nc.tensor.matmul(
                            psum[ob],
                            wt[:, wi, ob * 128 : (ob + 1) * 128],
                            basis[di],
                            start=(ib == 0 and di == 0),
                            stop=(ib == IB - 1 and di == DEG - 1),
                        )

                if bias_tiles is None:
                    # emitted late: off the kernel-head critical path
                    bias_tiles = []
                    for ob in range(OB):
                        bt = biaspool.tile([128, 1], f32, tag="bias")
                        nc.sync.dma_start(out=bt, in_=bias_ext[ob])
                        bias_tiles.append(bt)

                # ---- drain: y = psum * 2^-14 + bias ----
                for ob in range(OB):
                    ot = opool.tile([128, TT], f32, tag="o")
                    nc.scalar.activation(
                        out=ot, in_=psum[ob], func=AF.Identity,
                        scale=float(1.0 / SCALE), bias=bias_tiles[ob],
                    )
                    nc.sync.dma_start(
                        out=yt_ext[ob * 128 : (ob + 1) * 128, tt * TT : (tt + 1) * TT],
                        in_=ot,
                    )

    nc.finalize()
    _CACHE["nc"] = nc
    return nc


def _prep_inputs(x, cheby_coeffs):
    x = np.asarray(x, dtype=np.float32)
    coeffs = np.asarray(cheby_coeffs, dtype=np.float32)

    bias = coeffs[:, :, 0].sum(axis=0).astype(np.float32)  # [NO]
    bias = np.ascontiguousarray(bias.reshape(OB, 128, 1))

    # wk[kt=(ib, d-1)][i_in][o] = coeffs[ib*128+i_in, o, d] * SCALE  (fp16)
    w = coeffs[:, :, 1:]  # [NI, NO, DEG]
    w = np.transpose(w.reshape(IB, 128, NO, DEG), (0, 3, 1, 2))  # [IB, DEG, 128, NO]
    wk = np.ascontiguousarray((w * SCALE).reshape(KT, 128, NO)).astype(np.float16)

    in_maps = []
    for c in range(N_CORES):
        xs = x[c * TOK_PER_CORE : (c + 1) * TOK_PER_CORE]  # [1024, NI]
        xt = np.ascontiguousarray(xs.T)  # [NI, 1024]
        in_maps.append({"xt": xt, "wk": wk, "bias": bias})
    return in_maps


def _gather(results):
    y = np.empty((N_TOKENS, NO), dtype=np.float32)
    for c in range(N_CORES):
        y[c * TOK_PER_CORE : (c + 1) * TOK_PER_CORE] = results[c]["yt"].T
    return y


def kernel(x, cheby_coeffs, _trace=False):
    from concourse.bass_utils import run_bass_kernel_spmd

    nc = _build()
    in_maps = _prep_inputs(x, cheby_coeffs)
    res = run_bass_kernel_spmd(
        nc, in_maps, list(range(N_CORES)), trace=_trace,
        **({"trace_cores": list(range(N_CORES))} if _trace else {}),
    )
    y = _gather(res.results)
    if _trace:
        return y, res
    return y
